# revision 11
# baseline (speedup 1.0000x reference)
"""GPTQ int4 quantized linear (CaiQuantLinear) on 8 Trainium2 NeuronCores.

y = x @ dequant(qweight, scales, qzeros) + bias
  x: [8192, 4096] f32, qweight: [256, 4096] int64 (16x 4-bit packed along
  infeatures), scales: [32, 4096] f32, qzeros: [32, 256] int64 (packed along
  outfeatures), g_idx = arange(4096)//128, bias: [4096] f32 -> y: [8192, 4096] f32

Sharding: 4 token-shards x 2 outfeature-shards = 8 cores. Core c handles
tokens [2048*(c//2), +2048) and outfeatures [2048*(c%2), +2048).

Device kernel (per core): weights ship as one byte per nibble with the
4-bit code in the HIGH bits (host bit-shuffle only), so dequant is two
tensor_tensor ops: (q_u8 - 16z) * (s/16), with scale/zero rows shipped
compact (262KB total) and replicated across partitions on-chip by 0-stride
partition-broadcast DMAs. Replication writes count against the ~420GB/s
DMA fabric like HBM reads do.

The load phase is fabric-bound: x (16.8MB) plus the first two weight sets
(2MB codes + 8MB replication) exceed what one outfeature block's 56us of
matmuls can cover. So ob0 and ob1 are computed TOGETHER as four b-outer
quartets (8 PSUM groups = 4 token-blocks x 2 obs): 112us of PE work whose
per-k-tile input demand (~240GB/s) stays under the fabric rate, with
token-blocks 0-3 shipped k-major so the b-loop consumes 32KB x-slices as
they arrive. Streams are issued in need-order, interleaved per 8-tile
octave across both HWDGE rings; later blocks prefetch in the quartets'
slack. Matmuls accumulate 32 k-tiles of [128,128]x[128,256] bf16 into
PSUM; evacuation adds the bias.
"""

import sys

if "/opt/trn_rl_repo" not in sys.path:
    sys.path.insert(0, "/opt/trn_rl_repo")

import numpy as np
import ml_dtypes

import concourse.bass as bass  # noqa: F401  (registers mybir types)
import concourse.mybir as mybir
import concourse.tile as tile
from concourse import bacc
from concourse.bass_utils import run_bass_kernel_spmd

BF16 = mybir.dt.bfloat16
F32 = mybir.dt.float32
U8 = mybir.dt.uint8

N_CORES = 8
NT, NO = 4, 2          # token shards x outfeature shards
TOK, IN_F, OUT_F = 8192, 4096, 4096
T = TOK // NT          # 2048 tokens per core
OS = OUT_F // NO       # 2048 outfeatures per core
P = 128
NB = IN_F // P         # 32 contraction super-tiles
OB = 256               # outfeature block (psum free dim)
NOB = OS // OB         # 8
NTB = T // P           # 16 token blocks
NQ = 4                 # token-blocks shipped k-major for the head quartets

CB = 8                 # super-tiles per weight-stream chunk
NCH = NB // CB         # 4 chunks per outfeature block
CBX = 4                # super-tiles per k-major x chunk
NCHX = NB // CBX       # 8 chunks for the k-major x stream

_CACHE = {}


def _build_program():
    nc = bacc.Bacc("TRN2", target_bir_lowering=False, debug=False,
                   num_devices=N_CORES)
    xq_ap = nc.dram_tensor("xq", [NCHX, P, CBX, NQ, P], BF16,
                           kind="ExternalInput").ap()
    xt_ap = nc.dram_tensor("xt", [NTB - NQ, P, NB, P], BF16,
                           kind="ExternalInput").ap()
    qs_ap = nc.dram_tensor("qs", [NOB, NCH, P, CB * OB], U8,
                           kind="ExternalInput").ap()
    sz_ap = nc.dram_tensor("sz", [NOB, NCH, CB, 2 * OB], BF16,
                           kind="ExternalInput").ap()
    br_ap = nc.dram_tensor("br", [OS], BF16, kind="ExternalInput").ap()
    y_ap = nc.dram_tensor("y", [NTB, NOB, P, OB], F32, kind="ExternalOutput").ap()

    with tile.TileContext(nc) as tc:
        with tc.tile_pool(name="resident", bufs=1) as rpool, \
             tc.tile_pool(name="wset", bufs=3) as wpool, \
             tc.tile_pool(name="qstream", bufs=2) as qpool, \
             tc.tile_pool(name="szstream", bufs=2) as szpool, \
             tc.tile_pool(name="ostream", bufs=4) as opool, \
             tc.tile_pool(name="psum", bufs=8, space="PSUM") as ppool:
            # bias replicate on the otherwise-idle gpsimd queue (needed
            # only by the first evacuation ~40us in)
            br_sb = rpool.tile([P, OS], BF16)
            nc.gpsimd.dma_start(br_sb[:], br_ap.partition_broadcast(P))
            # zeros rhs for PE-warmup matmuls during the load phase
            wz = rpool.tile([P, OB], BF16)
            nc.gpsimd.memset(wz[:], 0.0)
            xq_sb = rpool.tile([P, NB, NQ, P], BF16)    # tb 0..3, k-major
            xt_sb = rpool.tile([P, NTB - NQ, NB, P], BF16)

            def lhsT(tb, b):
                if tb < NQ:
                    return xq_sb[:, b, tb, :]
                return xt_sb[:, tb - NQ, b, :]

            # junk psum tile for PE warmup; rotation hands it to the last
            # quartet group once the junk matmuls are done
            js = ppool.tile([P, OB], F32, tag="ps", name="js")
            for _ in range(2):
                nc.tensor.matmul(js[:], wz[:, :P], wz[:], start=True, stop=True)

            def dequant(wset, q_sb, sz_sb, ch):
                for l in range(CB):
                    b = ch * CB + l
                    tmp = qpool.tile([P, OB], BF16, tag="tmp")
                    nc.vector.tensor_tensor(
                        tmp[:], q_sb[:, l * OB:(l + 1) * OB],
                        sz_sb[:, l, OB:], mybir.AluOpType.subtract)
                    nc.vector.tensor_tensor(
                        wset[:, b, :], tmp[:], sz_sb[:, l, :OB],
                        mybir.AluOpType.mult)

            # --- head streams, need-order: per contraction-octave, ship
            # ob0's and ob1's weight chunk + replication + the two k-major
            # x chunks, one ob per ring; then the token-major x tiles
            wsets = [wpool.tile([P, NB, OB], BF16, tag="wset", name=f"w{o}")
                     for o in range(2)]
            for ch in range(NCH):
                for o in range(2):
                    eng = nc.sync if o == 0 else nc.scalar
                    q_sb = qpool.tile([P, CB * OB], U8, tag="q")
                    eng.dma_start(q_sb[:], qs_ap[o, ch])
                    sz_sb = szpool.tile([P, CB, 2 * OB], BF16, tag="sz")
                    eng.dma_start(sz_sb[:], sz_ap[o, ch].partition_broadcast(P))
                    if ch == 0:
                        # junk matmul on arrived bytes keeps the PE p-state
                        # ramping before the first dequanted weights exist
                        nc.tensor.matmul(
                            js[:], q_sb[:, :2 * P].bitcast(BF16), wz[:],
                            start=True, stop=True)
                    dequant(wsets[o], q_sb, sz_sb, ch)
                for i in range(2):
                    cx = 2 * ch + i
                    eng = nc.sync if i == 0 else nc.scalar
                    eng.dma_start(xq_sb[:, CBX * cx:CBX * (cx + 1)], xq_ap[cx])
            for tb in range(NQ, NTB):
                eng = nc.scalar if tb % 2 else nc.sync
                eng.dma_start(xt_sb[:, tb - NQ], xt_ap[tb - NQ])

            def evac(pslice, tb, ob):
                ot = opool.tile([P, OB], F32, tag="ot")
                nc.vector.tensor_tensor(
                    ot[:], pslice, br_sb[:, ob * OB:(ob + 1) * OB],
                    mybir.AluOpType.add)
                nc.gpsimd.dma_start(y_ap[tb, ob], ot[:])

            def produce_wset(ob):
                wset = wpool.tile([P, NB, OB], BF16, tag="wset")
                for ch in range(NCH):
                    q_sb = qpool.tile([P, CB * OB], U8, tag="q")
                    nc.sync.dma_start(q_sb[:], qs_ap[ob, ch])
                    sz_sb = szpool.tile([P, CB, 2 * OB], BF16, tag="sz")
                    nc.scalar.dma_start(sz_sb[:],
                                        sz_ap[ob, ch].partition_broadcast(P))
                    dequant(wset, q_sb, sz_sb, ch)
                return wset

            def quartet(qd):
                pst = [ppool.tile([P, OB], F32, tag="ps",
                                  name=f"p{qd}_{g}") for g in range(2 * NQ)]
                for b in range(NB):
                    for g in range(2 * NQ):
                        tb, o = qd * NQ + g % NQ, g // NQ
                        nc.tensor.matmul(
                            pst[g][:], lhsT(tb, b), wsets[o][:, b, :],
                            start=(b == 0), stop=(b == NB - 1))
                for g in range(2 * NQ):
                    evac(pst[g][:], qd * NQ + g % NQ, g // NQ)

            # paired head: ob0+ob1 over four quartets; later blocks
            # prefetch in the quartets' stream slack
            quartet(0)
            quartet(1)
            wset2 = produce_wset(2)
            quartet(2)
            wset3 = produce_wset(3)
            quartet(3)

            nexts = [wset2, wset3]
            for ob in range(2, NOB):
                wset = nexts[ob - 2]
                if ob + 2 < NOB:
                    nexts.append(produce_wset(ob + 2))
                for tb in range(NTB):
                    ps = ppool.tile([P, OB], F32, tag="ps")
                    for b in range(NB):
                        nc.tensor.matmul(
                            ps[:], lhsT(tb, b), wset[:, b, :],
                            start=(b == 0), stop=(b == NB - 1))
                    evac(ps[:], tb, ob)

    nc.compile()
    return nc


def _host_prep(x, qweight, scales, qzeros, bias):
    """Per-core input maps: layout prep only (transpose / nibble byte-split);
    dequantization (zero-subtract, scale-multiply) happens on-chip."""
    bf16 = ml_dtypes.bfloat16
    x = np.asarray(x, dtype=np.float32)
    qw = np.asarray(qweight).astype(np.int64, copy=False)
    sc = np.asarray(scales, dtype=np.float32)
    qz = np.asarray(qzeros).astype(np.int64, copy=False)
    bi = np.asarray(bias, dtype=np.float32)

    # zeros: unpack along outfeatures, +1 (pack() stored z-1)
    shifts = (np.arange(16, dtype=np.uint64) * np.uint64(4))
    zz = ((qz.astype(np.uint64)[:, :, None] >> shifts[None, None, :])
          & np.uint64(15)).reshape(qz.shape[0], -1).astype(np.float32) + 1.0

    # per-token-shard xT: tb 0..3 k-major [NCHX, P, CBX, NQ, P];
    # tb 4..15 token-major [NTB-NQ, P, NB, P]
    xq_list, xt_list = [], []
    for tc in range(NT):
        xs = x[tc * T:(tc + 1) * T]                      # [T, IN_F]
        xt = np.ascontiguousarray(xs.T).astype(bf16)     # [IN_F, T]
        xt4 = xt.reshape(NB, P, NTB, P).transpose(2, 1, 0, 3)  # [tb, p, b, t]
        xq = np.ascontiguousarray(
            xt4[:NQ].transpose(2, 1, 0, 3)               # [b, p, tb, t]
               .reshape(NCHX, CBX, P, NQ, P).transpose(0, 2, 1, 3, 4))
        xq_list.append(xq)
        xt_list.append(np.ascontiguousarray(xt4[NQ:]))

    # per-outfeature-shard weight-side tensors (shared by NT cores)
    qs_list, sz_list, br_list = [], [], []
    for oc in range(NO):
        o0 = oc * OS
        qsl = np.ascontiguousarray(qw[:, o0:o0 + OS])    # [256, OS] int64
        qbytes = qsl.view(np.uint8).reshape(IN_F // 16, OS, 8)
        qb2 = np.ascontiguousarray(qbytes.transpose(0, 2, 1)).reshape(IN_F // 2, OS)
        nib = np.empty((IN_F, OS), np.uint8)             # row k: code(k, o) << 4
        nib[0::2] = (qb2 & np.uint8(15)) << np.uint8(4)
        nib[1::2] = qb2 & np.uint8(0xF0)
        qs_t = np.ascontiguousarray(
            nib.reshape(NCH, CB, P, NOB, OB).transpose(3, 0, 2, 1, 4)
               .reshape(NOB, NCH, P, CB * OB))
        qs_list.append(qs_t)

        s16 = (sc[:, o0:o0 + OS] / 16.0).astype(bf16).reshape(NB, NOB, OB)
        z16 = (zz[:, o0:o0 + OS] * 16.0).astype(bf16).reshape(NB, NOB, OB)
        sz = np.concatenate([s16, z16], axis=-1)         # [NB, NOB, 2*OB]
        sz_t = np.ascontiguousarray(
            sz.reshape(NCH, CB, NOB, 2 * OB).transpose(2, 0, 1, 3))
        sz_list.append(sz_t)                             # [NOB, NCH, CB, 2*OB]
        br_list.append(np.ascontiguousarray(bi[o0:o0 + OS].astype(bf16)))

    in_maps = []
    for c in range(N_CORES):
        tc, oc = c // NO, c % NO
        in_maps.append({
            "xq": xq_list[tc],
            "xt": xt_list[tc],
            "qs": qs_list[oc],
            "sz": sz_list[oc],
            "br": br_list[oc],
        })
    return in_maps


def get_program():
    if "nc" not in _CACHE:
        _CACHE["nc"] = _build_program()
    return _CACHE["nc"]


def kernel(x, qweight, scales, qzeros, g_idx, bias):
    nc = get_program()
    in_maps = _host_prep(x, qweight, scales, qzeros, bias)
    res = run_bass_kernel_spmd(nc, in_maps, core_ids=list(range(N_CORES)))
    y = np.empty((TOK, OUT_F), dtype=np.float32)
    for c in range(N_CORES):
        tc, oc = c // NO, c % NO
        yt = res.results[c]["y"]                         # [NTB, NOB, P, OB]
        y[tc * T:(tc + 1) * T, oc * OS:(oc + 1) * OS] = (
            yt.transpose(0, 2, 1, 3).reshape(T, OS))
    return y


# revision 13
# speedup vs baseline: 1.0040x; 1.0040x over previous
"""GPTQ int4 quantized linear (CaiQuantLinear) on 8 Trainium2 NeuronCores.

y = x @ dequant(qweight, scales, qzeros) + bias
  x: [8192, 4096] f32, qweight: [256, 4096] int64 (16x 4-bit packed along
  infeatures), scales: [32, 4096] f32, qzeros: [32, 256] int64 (packed along
  outfeatures), g_idx = arange(4096)//128, bias: [4096] f32 -> y: [8192, 4096] f32

Sharding: 4 token-shards x 2 outfeature-shards = 8 cores. Core c handles
tokens [2048*(c//2), +2048) and outfeatures [2048*(c%2), +2048).

Device kernel (per core): weights ship as one byte per nibble with the
4-bit code in the HIGH bits (host bit-shuffle only); scale/zero rows ship
compact (262KB total), ordered [all s | all z] per chunk, and are
replicated across partitions on-chip by 0-stride partition-broadcast DMAs.
Dequant is then just TWO whole-chunk tensor_tensor ops (in-place:
wset = (q_u8 - 16z) * (s/16)), so the DVE never rate-limits the stream.

The load phase is fabric-bound (~420GB/s aggregate, replication writes
included): x (16.8MB) + the first two weight sets exceed one block's 56us
of matmuls. ob0 and ob1 are therefore computed TOGETHER as four b-outer
quartets (8 PSUM groups = 4 token-blocks x 2 obs): 112us of PE work whose
per-k-tile demand (~240GB/s) the fabric can sustain, with token-blocks 0-3
shipped k-major so the b-loop consumes 32KB x-slices as they arrive.
Matmuls accumulate 32 k-tiles of [128,128]x[128,256] bf16 into PSUM;
evacuation adds the bias.
"""

import sys

if "/opt/trn_rl_repo" not in sys.path:
    sys.path.insert(0, "/opt/trn_rl_repo")

import numpy as np
import ml_dtypes

import concourse.bass as bass  # noqa: F401  (registers mybir types)
import concourse.mybir as mybir
import concourse.tile as tile
from concourse import bacc
from concourse.bass_utils import run_bass_kernel_spmd

BF16 = mybir.dt.bfloat16
F32 = mybir.dt.float32
U8 = mybir.dt.uint8

N_CORES = 8
NT, NO = 4, 2          # token shards x outfeature shards
TOK, IN_F, OUT_F = 8192, 4096, 4096
T = TOK // NT          # 2048 tokens per core
OS = OUT_F // NO       # 2048 outfeatures per core
P = 128
NB = IN_F // P         # 32 contraction super-tiles
OB = 256               # outfeature block (psum free dim)
NOB = OS // OB         # 8
NTB = T // P           # 16 token blocks
NQ = 4                 # token-blocks shipped k-major for the head quartets

CB = 8                 # super-tiles per weight-stream chunk
NCH = NB // CB         # 4 chunks per outfeature block
CBX = 4                # super-tiles per k-major x chunk
NCHX = NB // CBX       # 8 chunks for the k-major x stream

_CACHE = {}


def _build_program():
    nc = bacc.Bacc("TRN2", target_bir_lowering=False, debug=False,
                   num_devices=N_CORES)
    xq_ap = nc.dram_tensor("xq", [NCHX, P, CBX, NQ, P], BF16,
                           kind="ExternalInput").ap()
    xt_ap = nc.dram_tensor("xt", [NTB - NQ, P, NB, P], BF16,
                           kind="ExternalInput").ap()
    qs_ap = nc.dram_tensor("qs", [NOB, NCH, P, CB, OB], U8,
                           kind="ExternalInput").ap()
    sz_ap = nc.dram_tensor("sz", [NOB, NCH, 2, CB, OB], BF16,
                           kind="ExternalInput").ap()
    br_ap = nc.dram_tensor("br", [OS], BF16, kind="ExternalInput").ap()
    y_ap = nc.dram_tensor("y", [NTB, NOB, P, OB], F32, kind="ExternalOutput").ap()

    with tile.TileContext(nc) as tc:
        with tc.tile_pool(name="resident", bufs=1) as rpool, \
             tc.tile_pool(name="wset", bufs=2) as wpool, \
             tc.tile_pool(name="qstream", bufs=4) as qpool, \
             tc.tile_pool(name="szstream", bufs=3) as szpool, \
             tc.tile_pool(name="ostream", bufs=6) as opool, \
             tc.tile_pool(name="psum", bufs=8, space="PSUM") as ppool:
            # bias replicate on the otherwise-idle gpsimd queue (needed
            # only by the first evacuation ~40us in)
            br_sb = rpool.tile([P, OS], BF16)
            nc.gpsimd.dma_start(br_sb[:], br_ap.partition_broadcast(P))
            # zeros rhs for PE-warmup matmuls during the load phase
            wz = rpool.tile([P, OB], BF16)
            nc.gpsimd.memset(wz[:], 0.0)
            xq_sb = rpool.tile([P, NB, NQ, P], BF16)    # tb 0..3, k-major
            xt_sb = rpool.tile([P, NTB - NQ, NB, P], BF16)

            def lhsT(tb, b):
                if tb < NQ:
                    return xq_sb[:, b, tb, :]
                return xt_sb[:, tb - NQ, b, :]

            # junk psum tile for PE warmup; rotation hands it to the last
            # quartet group once the junk matmuls are done
            js = ppool.tile([P, OB], F32, tag="ps", name="js")
            for _ in range(2):
                nc.tensor.matmul(js[:], wz[:, :P], wz[:], start=True, stop=True)

            def dequant(wset, q_sb, sz_sb, ch):
                dst = wset[:, ch * CB:(ch + 1) * CB, :]
                nc.vector.tensor_tensor(
                    dst, q_sb[:], sz_sb[:, 1], mybir.AluOpType.subtract)
                nc.vector.tensor_tensor(
                    dst, dst, sz_sb[:, 0], mybir.AluOpType.mult)

            # --- head streams, need-order: per contraction-octave, ship
            # ob0's and ob1's weight chunk + replication + the two k-major
            # x chunks, one ob per ring; then the token-major x tiles
            wsets = [wpool.tile([P, NB, OB], BF16, tag="wset", name=f"w{o}")
                     for o in range(2)]
            for ch in range(NCH):
                for o in range(2):
                    eng = nc.sync if o == 0 else nc.scalar
                    q_sb = qpool.tile([P, CB, OB], U8, tag="q")
                    eng.dma_start(q_sb[:], qs_ap[o, ch])
                    sz_sb = szpool.tile([P, 2, CB, OB], BF16, tag="sz")
                    eng.dma_start(sz_sb[:], sz_ap[o, ch].partition_broadcast(P))
                    if ch == 0:
                        # junk matmul on arrived bytes keeps the PE p-state
                        # ramping before the first dequanted weights exist
                        nc.tensor.matmul(
                            js[:], q_sb[:, 0, :2 * P].bitcast(BF16), wz[:],
                            start=True, stop=True)
                    dequant(wsets[o], q_sb, sz_sb, ch)
                for i in range(2):
                    cx = 2 * ch + i
                    eng = nc.sync if i == 0 else nc.scalar
                    eng.dma_start(xq_sb[:, CBX * cx:CBX * (cx + 1)], xq_ap[cx])
            for tb in range(NQ, NTB):
                eng = nc.scalar if tb % 2 else nc.sync
                eng.dma_start(xt_sb[:, tb - NQ], xt_ap[tb - NQ])

            def evac(pslice, tb, ob):
                ot = opool.tile([P, OB], F32, tag="ot")
                nc.vector.tensor_tensor(
                    ot[:], pslice, br_sb[:, ob * OB:(ob + 1) * OB],
                    mybir.AluOpType.add)
                nc.gpsimd.dma_start(y_ap[tb, ob], ot[:])

            def produce_wset(ob):
                wset = wpool.tile([P, NB, OB], BF16, tag="wset")
                for ch in range(NCH):
                    q_sb = qpool.tile([P, CB, OB], U8, tag="q")
                    nc.sync.dma_start(q_sb[:], qs_ap[ob, ch])
                    sz_sb = szpool.tile([P, 2, CB, OB], BF16, tag="sz")
                    nc.scalar.dma_start(sz_sb[:],
                                        sz_ap[ob, ch].partition_broadcast(P))
                    dequant(wset, q_sb, sz_sb, ch)
                return wset

            def quartet(qd):
                pst = [ppool.tile([P, OB], F32, tag="ps",
                                  name=f"p{qd}_{g}") for g in range(2 * NQ)]
                for b in range(NB):
                    for g in range(2 * NQ):
                        tb, o = qd * NQ + g % NQ, g // NQ
                        nc.tensor.matmul(
                            pst[g][:], lhsT(tb, b), wsets[o][:, b, :],
                            start=(b == 0), stop=(b == NB - 1))
                for g in range(2 * NQ):
                    evac(pst[g][:], qd * NQ + g % NQ, g // NQ)

            # paired head: ob0+ob1 over four quartets
            for qd in range(NTB // NQ):
                quartet(qd)

            for ob in range(2, NOB):
                wset = produce_wset(ob)
                for tb in range(NTB):
                    ps = ppool.tile([P, OB], F32, tag="ps")
                    for b in range(NB):
                        nc.tensor.matmul(
                            ps[:], lhsT(tb, b), wset[:, b, :],
                            start=(b == 0), stop=(b == NB - 1))
                    evac(ps[:], tb, ob)

    nc.compile()
    return nc


def _host_prep(x, qweight, scales, qzeros, bias):
    """Per-core input maps: layout prep only (transpose / nibble byte-split);
    dequantization (zero-subtract, scale-multiply) happens on-chip."""
    bf16 = ml_dtypes.bfloat16
    x = np.asarray(x, dtype=np.float32)
    qw = np.asarray(qweight).astype(np.int64, copy=False)
    sc = np.asarray(scales, dtype=np.float32)
    qz = np.asarray(qzeros).astype(np.int64, copy=False)
    bi = np.asarray(bias, dtype=np.float32)

    # zeros: unpack along outfeatures, +1 (pack() stored z-1)
    shifts = (np.arange(16, dtype=np.uint64) * np.uint64(4))
    zz = ((qz.astype(np.uint64)[:, :, None] >> shifts[None, None, :])
          & np.uint64(15)).reshape(qz.shape[0], -1).astype(np.float32) + 1.0

    # per-token-shard xT: tb 0..3 k-major [NCHX, P, CBX, NQ, P];
    # tb 4..15 token-major [NTB-NQ, P, NB, P]
    xq_list, xt_list = [], []
    for tc in range(NT):
        xs = x[tc * T:(tc + 1) * T]                      # [T, IN_F]
        xt = np.ascontiguousarray(xs.T).astype(bf16)     # [IN_F, T]
        xt4 = xt.reshape(NB, P, NTB, P).transpose(2, 1, 0, 3)  # [tb, p, b, t]
        xq = np.ascontiguousarray(
            xt4[:NQ].transpose(2, 1, 0, 3)               # [b, p, tb, t]
               .reshape(NCHX, CBX, P, NQ, P).transpose(0, 2, 1, 3, 4))
        xq_list.append(xq)
        xt_list.append(np.ascontiguousarray(xt4[NQ:]))

    # per-outfeature-shard weight-side tensors (shared by NT cores)
    qs_list, sz_list, br_list = [], [], []
    for oc in range(NO):
        o0 = oc * OS
        qsl = np.ascontiguousarray(qw[:, o0:o0 + OS])    # [256, OS] int64
        qbytes = qsl.view(np.uint8).reshape(IN_F // 16, OS, 8)
        qb2 = np.ascontiguousarray(qbytes.transpose(0, 2, 1)).reshape(IN_F // 2, OS)
        nib = np.empty((IN_F, OS), np.uint8)             # row k: code(k, o) << 4
        nib[0::2] = (qb2 & np.uint8(15)) << np.uint8(4)
        nib[1::2] = qb2 & np.uint8(0xF0)
        qs_t = np.ascontiguousarray(
            nib.reshape(NCH, CB, P, NOB, OB).transpose(3, 0, 2, 1, 4))
        qs_list.append(qs_t)                             # [NOB, NCH, P, CB, OB]

        s16 = (sc[:, o0:o0 + OS] / 16.0).astype(bf16).reshape(NB, NOB, OB)
        z16 = (zz[:, o0:o0 + OS] * 16.0).astype(bf16).reshape(NB, NOB, OB)
        s_t = s16.reshape(NCH, CB, NOB, OB).transpose(2, 0, 1, 3)
        z_t = z16.reshape(NCH, CB, NOB, OB).transpose(2, 0, 1, 3)
        sz_list.append(np.ascontiguousarray(
            np.stack([s_t, z_t], axis=2)))               # [NOB, NCH, 2, CB, OB]
        br_list.append(np.ascontiguousarray(bi[o0:o0 + OS].astype(bf16)))

    in_maps = []
    for c in range(N_CORES):
        tc, oc = c // NO, c % NO
        in_maps.append({
            "xq": xq_list[tc],
            "xt": xt_list[tc],
            "qs": qs_list[oc],
            "sz": sz_list[oc],
            "br": br_list[oc],
        })
    return in_maps


def get_program():
    if "nc" not in _CACHE:
        _CACHE["nc"] = _build_program()
    return _CACHE["nc"]


def kernel(x, qweight, scales, qzeros, g_idx, bias):
    nc = get_program()
    in_maps = _host_prep(x, qweight, scales, qzeros, bias)
    res = run_bass_kernel_spmd(nc, in_maps, core_ids=list(range(N_CORES)))
    y = np.empty((TOK, OUT_F), dtype=np.float32)
    for c in range(N_CORES):
        tc, oc = c // NO, c % NO
        yt = res.results[c]["y"]                         # [NTB, NOB, P, OB]
        y[tc * T:(tc + 1) * T, oc * OS:(oc + 1) * OS] = (
            yt.transpose(0, 2, 1, 3).reshape(T, OS))
    return y


# revision 14
# speedup vs baseline: 1.0389x; 1.0347x over previous
"""GPTQ int4 quantized linear (CaiQuantLinear) on 8 Trainium2 NeuronCores.

y = x @ dequant(qweight, scales, qzeros) + bias
  x: [8192, 4096] f32, qweight: [256, 4096] int64 (16x 4-bit packed along
  infeatures), scales: [32, 4096] f32, qzeros: [32, 256] int64 (packed along
  outfeatures), g_idx = arange(4096)//128, bias: [4096] f32 -> y: [8192, 4096] f32

Sharding: 4 token-shards x 2 outfeature-shards = 8 cores. Core c handles
tokens [2048*(c//2), +2048) and outfeatures [2048*(c%2), +2048).

Device kernel (per core): weights ship as one byte per nibble with the
4-bit code in the HIGH bits (host bit-shuffle only); scale/zero rows ship
compact (262KB total), ordered [all s | all z] per chunk, and are
replicated across the 128 partitions on-chip by 0-stride partition-
broadcast DMAs on the two HWDGE rings. Dequant is two whole-chunk
tensor_tensor ops (in-place: wset = (q_u8 - 16z) * (s/16)), so the weight
stream is ~1MB + 4MB of replication writes per 2048x256 block instead of
50MB, and the DVE never rate-limits it. The x shard streams token-major,
striped across both rings, while each outfeature block's 512 matmuls
([128,128]x[128,256] bf16, 32 k-tiles accumulated in PSUM) cover the next
block's stream. Evacuation adds the bias.
"""

import sys

if "/opt/trn_rl_repo" not in sys.path:
    sys.path.insert(0, "/opt/trn_rl_repo")

import numpy as np
import ml_dtypes

import concourse.bass as bass  # noqa: F401  (registers mybir types)
import concourse.mybir as mybir
import concourse.tile as tile
from concourse import bacc
from concourse.bass_utils import run_bass_kernel_spmd

BF16 = mybir.dt.bfloat16
F32 = mybir.dt.float32
U8 = mybir.dt.uint8

N_CORES = 8
NT, NO = 4, 2          # token shards x outfeature shards
TOK, IN_F, OUT_F = 8192, 4096, 4096
T = TOK // NT          # 2048 tokens per core
OS = OUT_F // NO       # 2048 outfeatures per core
P = 128
NB = IN_F // P         # 32 contraction super-tiles
OB = 256               # outfeature block (psum free dim)
NOB = OS // OB         # 8
NTB = T // P           # 16 token blocks

CB = 8                 # super-tiles per weight-stream chunk
NCH = NB // CB         # 4 chunks per outfeature block

_CACHE = {}


def _build_program():
    nc = bacc.Bacc("TRN2", target_bir_lowering=False, debug=False,
                   num_devices=N_CORES)
    xt_ap = nc.dram_tensor("xt", [NTB, P, NB, P], BF16, kind="ExternalInput").ap()
    qs_ap = nc.dram_tensor("qs", [NOB, NCH, P, CB, OB], U8,
                           kind="ExternalInput").ap()
    sz_ap = nc.dram_tensor("sz", [NOB, NCH, 2, CB, OB], BF16,
                           kind="ExternalInput").ap()
    br_ap = nc.dram_tensor("br", [OS], BF16, kind="ExternalInput").ap()
    y_ap = nc.dram_tensor("y", [NTB, NOB, P, OB], F32, kind="ExternalOutput").ap()

    with tile.TileContext(nc) as tc:
        with tc.tile_pool(name="resident", bufs=1) as rpool, \
             tc.tile_pool(name="wset", bufs=2) as wpool, \
             tc.tile_pool(name="qstream", bufs=4) as qpool, \
             tc.tile_pool(name="szstream", bufs=3) as szpool, \
             tc.tile_pool(name="ostream", bufs=6) as opool, \
             tc.tile_pool(name="psum", bufs=8, space="PSUM") as ppool:
            # bias replicate on the otherwise-idle gpsimd queue (needed
            # only by the first evacuation ~45us in)
            br_sb = rpool.tile([P, OS], BF16)
            nc.gpsimd.dma_start(br_sb[:], br_ap.partition_broadcast(P))
            # zeros rhs for PE-warmup matmuls during the load phase
            wz = rpool.tile([P, OB], BF16)
            nc.gpsimd.memset(wz[:], 0.0)
            xt_sb = rpool.tile([P, NTB, NB, P], BF16)

            # junk psum tile for PE warmup; returns to the pool afterwards
            js = ppool.tile([P, OB], F32, tag="ps", name="js")
            for _ in range(2):
                nc.tensor.matmul(js[:], wz[:, :P], wz[:], start=True, stop=True)

            def dequant(wset, q_sb, sz_sb, ch):
                dst = wset[:, ch * CB:(ch + 1) * CB, :]
                nc.vector.tensor_tensor(
                    dst, q_sb[:], sz_sb[:, 1], mybir.AluOpType.subtract)
                nc.vector.tensor_tensor(
                    dst, dst, sz_sb[:, 0], mybir.AluOpType.mult)

            def produce_wset(ob, warm=False):
                wset = wpool.tile([P, NB, OB], BF16, tag="wset")
                for ch in range(NCH):
                    q_sb = qpool.tile([P, CB, OB], U8, tag="q")
                    nc.sync.dma_start(q_sb[:], qs_ap[ob, ch])
                    sz_sb = szpool.tile([P, 2, CB, OB], BF16, tag="sz")
                    # warm: balance the replication writes across both
                    # HWDGE rings (gpsimd's SWDGE is ~3x slower)
                    eng = nc.sync if (warm and ch % 2) else nc.scalar
                    eng.dma_start(sz_sb[:], sz_ap[ob, ch].partition_broadcast(P))
                    if warm:
                        # junk matmul on arrived bytes keeps the PE p-state
                        # ramping before the first dequanted weights exist
                        nc.tensor.matmul(
                            js[:], q_sb[:, 0, :2 * P].bitcast(BF16), wz[:],
                            start=True, stop=True)
                    dequant(wset, q_sb, sz_sb, ch)
                return wset

            # x token-blocks 0/1 lead; the ob0 weight stream rides between
            nc.sync.dma_start(xt_sb[:, 0], xt_ap[0])
            nc.scalar.dma_start(xt_sb[:, 1], xt_ap[1])
            wset = produce_wset(0, warm=True)
            for tb in range(2, NTB):
                eng = nc.scalar if tb % 2 else nc.sync
                eng.dma_start(xt_sb[:, tb], xt_ap[tb])
            for tb in range(2):
                nc.tensor.matmul(js[:], xt_sb[:, tb, 0, :], wz[:],
                                 start=True, stop=True)

            def evac(pslice, tb, ob):
                ot = opool.tile([P, OB], F32, tag="ot")
                nc.vector.tensor_tensor(
                    ot[:], pslice, br_sb[:, ob * OB:(ob + 1) * OB],
                    mybir.AluOpType.add)
                nc.gpsimd.dma_start(y_ap[tb, ob], ot[:])

            for ob in range(NOB):
                if ob > 0:
                    wset = produce_wset(ob)
                for tb in range(NTB):
                    ps = ppool.tile([P, OB], F32, tag="ps")
                    for b in range(NB):
                        nc.tensor.matmul(
                            ps[:], xt_sb[:, tb, b, :], wset[:, b, :],
                            start=(b == 0), stop=(b == NB - 1))
                    evac(ps[:], tb, ob)

    nc.compile()
    return nc


def _host_prep(x, qweight, scales, qzeros, bias):
    """Per-core input maps: layout prep only (transpose / nibble byte-split);
    dequantization (zero-subtract, scale-multiply) happens on-chip."""
    bf16 = ml_dtypes.bfloat16
    x = np.asarray(x, dtype=np.float32)
    qw = np.asarray(qweight).astype(np.int64, copy=False)
    sc = np.asarray(scales, dtype=np.float32)
    qz = np.asarray(qzeros).astype(np.int64, copy=False)
    bi = np.asarray(bias, dtype=np.float32)

    # zeros: unpack along outfeatures, +1 (pack() stored z-1)
    shifts = (np.arange(16, dtype=np.uint64) * np.uint64(4))
    zz = ((qz.astype(np.uint64)[:, :, None] >> shifts[None, None, :])
          & np.uint64(15)).reshape(qz.shape[0], -1).astype(np.float32) + 1.0

    # per-token-shard xT: [NTB, P(k-part), NB, P(t)]
    xt_list = []
    for tc in range(NT):
        xs = x[tc * T:(tc + 1) * T]                      # [T, IN_F]
        xt = np.ascontiguousarray(xs.T).astype(bf16)     # [IN_F, T]
        xt_list.append(np.ascontiguousarray(
            xt.reshape(NB, P, NTB, P).transpose(2, 1, 0, 3)))

    # per-outfeature-shard weight-side tensors (shared by NT cores)
    qs_list, sz_list, br_list = [], [], []
    for oc in range(NO):
        o0 = oc * OS
        qsl = np.ascontiguousarray(qw[:, o0:o0 + OS])    # [256, OS] int64
        qbytes = qsl.view(np.uint8).reshape(IN_F // 16, OS, 8)
        qb2 = np.ascontiguousarray(qbytes.transpose(0, 2, 1)).reshape(IN_F // 2, OS)
        nib = np.empty((IN_F, OS), np.uint8)             # row k: code(k, o) << 4
        nib[0::2] = (qb2 & np.uint8(15)) << np.uint8(4)
        nib[1::2] = qb2 & np.uint8(0xF0)
        qs_list.append(np.ascontiguousarray(
            nib.reshape(NCH, CB, P, NOB, OB).transpose(3, 0, 2, 1, 4)))

        s16 = (sc[:, o0:o0 + OS] / 16.0).astype(bf16).reshape(NB, NOB, OB)
        z16 = (zz[:, o0:o0 + OS] * 16.0).astype(bf16).reshape(NB, NOB, OB)
        s_t = s16.reshape(NCH, CB, NOB, OB).transpose(2, 0, 1, 3)
        z_t = z16.reshape(NCH, CB, NOB, OB).transpose(2, 0, 1, 3)
        sz_list.append(np.ascontiguousarray(
            np.stack([s_t, z_t], axis=2)))               # [NOB, NCH, 2, CB, OB]
        br_list.append(np.ascontiguousarray(bi[o0:o0 + OS].astype(bf16)))

    in_maps = []
    for c in range(N_CORES):
        tc, oc = c // NO, c % NO
        in_maps.append({
            "xt": xt_list[tc],
            "qs": qs_list[oc],
            "sz": sz_list[oc],
            "br": br_list[oc],
        })
    return in_maps


def get_program():
    if "nc" not in _CACHE:
        _CACHE["nc"] = _build_program()
    return _CACHE["nc"]


def kernel(x, qweight, scales, qzeros, g_idx, bias):
    nc = get_program()
    in_maps = _host_prep(x, qweight, scales, qzeros, bias)
    res = run_bass_kernel_spmd(nc, in_maps, core_ids=list(range(N_CORES)))
    y = np.empty((TOK, OUT_F), dtype=np.float32)
    for c in range(N_CORES):
        tc, oc = c // NO, c % NO
        yt = res.results[c]["y"]                         # [NTB, NOB, P, OB]
        y[tc * T:(tc + 1) * T, oc * OS:(oc + 1) * OS] = (
            yt.transpose(0, 2, 1, 3).reshape(T, OS))
    return y
